# revision 47
# baseline (speedup 1.0000x reference)
"""DGCN kernel for Trainium2 (8 NeuronCores, data-parallel over batch).

Reference computation (per sample):
  h   = conv1x1(x)                                   # [C,N,T]
  hsum= h.sum(T)                                     # = W @ x.sum(T) + T*b
  a1  = softmax(relu(hsum.T @ memory * s))           # [N,N]
  a2  = softmax(relu(hsum.T @ hsum * s))             # [N,N]
  adj = softmax(fc_w0*a1 + fc_w1*a2 + fc_b)          # [N,N]
  adj = topk_mask(adj, K) * adj                      # keep K largest/row,
                                                     # ties -> lowest index
  g1  = h  (.) adj ; g2 = g1 (.) adj                 # node contraction
  z   = gcn_w @ [g1;g2] + gcn_b
  out = z*emb + x

Top-k trick: the softmax rows have a huge tie group at the "floor" value
(entries whose relus are all exactly 0 collapse to one float). The K-th
largest always lands inside it, so the threshold equals the floor value,
which we compute EXACTLY by pushing a virtual 884th zero-padded column
through the identical pipeline (zero rhs column -> s=0 -> relu=0). The
mask is then
  (v > thr) | (v == thr & prefix_count(v == thr) <= K - count(v > thr))
which reproduces jax.lax.top_k's lowest-index-first tie breaking.

Precision: adjacency pipeline (hsum/s1/s2) in strict fp32 matmuls;
conv / diffusion / projection operands in bf16 with f32 PSUM accumulation.
"""
import math

import ml_dtypes
import numpy as np

import concourse.bass as bass
import concourse.mybir as mybir
import concourse.tile as tile
from concourse import bacc
from concourse.bass_utils import run_bass_kernel_spmd
from concourse.masks import make_identity

B, C, N, T = 32, 128, 883, 12
K = int(N * 0.8)  # 706
NCORES = 8
SPC = B // NCORES  # samples per core
SCALE = 1.0 / math.sqrt(C)
F32 = mybir.dt.float32
BF16 = mybir.dt.bfloat16
AX = mybir.AxisListType
OP = mybir.AluOpType
ACTF = mybir.ActivationFunctionType

NCH = (N + 127) // 128  # 7 node chunks
CH = [(j * 128, min(128, N - j * 128)) for j in range(NCH)]  # (start, size)
# free-dim chunks for the (N+1)-wide adjacency matmuls; col N is the
# zero-padded "virtual" column that carries the tie-group threshold.
MCH = [(0, 512), (512, 372)]
CT = C * T  # 1536


def _fch(total, step=512):
    return [(f, min(step, total - f)) for f in range(0, total, step)]


def build_nc():
    nc = bacc.Bacc(None)
    x_d = nc.dram_tensor("x", [SPC, C, N, T], F32, kind="ExternalInput")
    y_d = nc.dram_tensor("y", [SPC, C, N, T], F32, kind="ExternalOutput")
    convwT_d = nc.dram_tensor("convwT", [C, C], F32, kind="ExternalInput")
    convwTb_d = nc.dram_tensor("convwTb", [C, C], BF16, kind="ExternalInput")
    convbp_d = nc.dram_tensor("convbp", [C, 1], F32, kind="ExternalInput")
    convb12p_d = nc.dram_tensor("convb12p", [C, 1], F32, kind="ExternalInput")
    memory_d = nc.dram_tensor("memory", [C, N], F32, kind="ExternalInput")
    fcw0_d = nc.dram_tensor("fcw0", [C, 1], F32, kind="ExternalInput")
    fcw1_d = nc.dram_tensor("fcw1", [C, 1], F32, kind="ExternalInput")
    fcb_d = nc.dram_tensor("fcb", [C, 1], F32, kind="ExternalInput")
    gw1T_d = nc.dram_tensor("gw1T", [C, C], BF16, kind="ExternalInput")
    gw2T_d = nc.dram_tensor("gw2T", [C, C], BF16, kind="ExternalInput")
    gcnbb_d = nc.dram_tensor("gcnbb", [1, C], BF16, kind="ExternalInput")
    emb_d = nc.dram_tensor("emb", [C, 1], F32, kind="ExternalInput")

    with tile.TileContext(nc) as tc:
        with (
            tc.tile_pool(name="const", bufs=1) as constp,
            tc.tile_pool(name="persist", bufs=1) as pers,
            tc.tile_pool(name="xin", bufs=2) as xinp,
            tc.tile_pool(name="hwin", bufs=2) as hwinp,
            tc.tile_pool(name="scr", bufs=15) as scrp,
            tc.tile_pool(name="col", bufs=8) as colp,
            tc.tile_pool(name="gcm", bufs=2) as gcmp,
            tc.tile_pool(name="outw", bufs=2) as outwp,
            tc.tile_pool(name="mmps", bufs=6, space=bass.MemorySpace.PSUM) as mmps,
            tc.tile_pool(name="tpps", bufs=2, space=bass.MemorySpace.PSUM) as tpps,
        ):
            # ---- constants / weights ----
            identb = constp.tile([128, 128], BF16)
            make_identity(nc, identb[:])
            zeros = constp.tile([128, N], F32)
            nc.gpsimd.memset(zeros[:], 0.0)
            memp = constp.tile([C, N + 1], F32)
            nc.sync.dma_start(memp[:, :N], memory_d[:])
            nc.gpsimd.memset(memp[:, N : N + 1], 0.0)
            convwT = constp.tile_from(convwT_d[:])
            convwTb = constp.tile_from(convwTb_d[:])
            convbp = constp.tile_from(convbp_d[:])
            convb12p = constp.tile_from(convb12p_d[:])
            fcw0 = constp.tile_from(fcw0_d[:])
            fcw1 = constp.tile_from(fcw1_d[:])
            fcb = constp.tile_from(fcb_d[:])
            gw1T = constp.tile_from(gw1T_d[:])
            gw2T = constp.tile_from(gw2T_d[:])
            gcnbb = constp.tile_from(gcnbb_d[:])
            embv = constp.tile_from(emb_d[:])
            onesb = constp.tile([1, 512], BF16)
            nc.gpsimd.memset(onesb[:], 1.0)

            for s in range(SPC):
                # ---- persistent per-sample tensors (t-major hT/g1T) ----
                hT = pers.tile([128, NCH, T, C], BF16, tag="hT")
                g1T = pers.tile([128, NCH, T, C], BF16, tag="g1T")
                g2T = pers.tile([128, NCH, T, C], BF16, tag="g2T")
                adjB = pers.tile([128, NCH, N + 1], BF16, tag="adjB")
                xsum = pers.tile([128, N + 1], F32, tag="xsum")
                hsum = pers.tile([128, N + 1], F32, tag="hsum")
                nc.vector.memset(xsum[:, N : N + 1], 0.0)

                xf = x_d[s].rearrange("c n t -> c (n t)")
                yf = y_d[s].rearrange("c n t -> c (n t)")

                # ========== stage A: conv (bf16) + transpose + xsum ========
                for j, (n0, sz) in enumerate(CH):
                    xw = xinp.tile([128, CT], F32, tag="xw")
                    nc.sync.dma_start(xw[:, : sz * T], xf[:, n0 * T : (n0 + sz) * T])
                    xv = xw[:, : sz * T].rearrange("p (n t) -> p n t", t=T)
                    nc.vector.tensor_reduce(
                        xsum[:, n0 : n0 + sz], xv, axis=AX.X, op=OP.add
                    )
                    # bf16 copy of the window for the conv matmul (DMA casts)
                    xb = hwinp.tile([128, CT], BF16, tag="xb")
                    nc.gpsimd.dma_start(
                        xb[:, : sz * T], xf[:, n0 * T : (n0 + sz) * T]
                    )
                    hw = hwinp.tile([128, CT], BF16, tag="hw")
                    for f0, fs in _fch(sz * T):
                        ps = mmps.tile([128, 512], F32, tag="mm")
                        nc.tensor.matmul(
                            ps[:, :fs], convwTb[:], xb[:, f0 : f0 + fs],
                            start=True, stop=True,
                        )
                        # h = Wx + b  (bias via per-partition scalar add)
                        nc.vector.tensor_scalar(
                            hw[:, f0 : f0 + fs], ps[:, :fs], convbp[:], None,
                            op0=OP.add,
                        )
                    # transpose h window -> hT[:, j] (batched eviction)
                    hv = hw[:, : sz * T].rearrange("p (n t) -> p n t", t=T)
                    for th in range(2):
                        tp = tpps.tile([128, T // 2, 128], BF16, tag="tp")
                        for tt in range(T // 2):
                            nc.tensor.transpose(
                                tp[:sz, tt, :], hv[:, :, th * 6 + tt], identb[:]
                            )
                        dst = hT[:sz, j, th * 6 : th * 6 + 6]
                        if (j + th) % 2 == 0:
                            nc.vector.tensor_copy(dst, tp[:sz])
                        else:
                            nc.scalar.activation(dst, tp[:sz], ACTF.Copy)

                # hsum = W @ xsum + T*conv_b   (strict fp32; virtual col -> 0
                # because xsum col N is 0 and bias is added only to real cols)
                for f0, fs in MCH:
                    ps = mmps.tile([128, 512], F32, tag="mm")
                    nc.tensor.matmul(
                        ps[:, :fs], convwT[:], xsum[:, f0 : f0 + fs],
                        start=True, stop=True,
                    )
                    real = min(fs, N - f0)  # exclude virtual col from bias
                    nc.vector.tensor_scalar(
                        hsum[:, f0 : f0 + real], ps[:, :real], convb12p[:], None,
                        op0=OP.add,
                    )
                    if real < fs:
                        nc.vector.tensor_copy(
                            hsum[:, f0 + real : f0 + fs], ps[:, real:fs]
                        )

                # ========== stage B: adjacency + exact top-k mask ==========
                for j, (n0, sz) in enumerate(CH):
                    lhs = hsum[:, n0 : n0 + sz]
                    r1 = scrp.tile([128, N + 1], F32, tag="scr")
                    r2 = scrp.tile([128, N + 1], F32, tag="scr")
                    for (f0, fs), rt, rhs in (
                        (MCH[0], r1, memp), (MCH[1], r1, memp),
                        (MCH[0], r2, hsum), (MCH[1], r2, hsum),
                    ):
                        ps = mmps.tile([128, 512], F32, tag="mm")
                        nc.tensor.matmul(
                            ps[:sz, :fs], lhs, rhs[:, f0 : f0 + fs],
                            start=True, stop=True,
                        )
                        # relu(s * scale) -- matches reference op order
                        nc.scalar.activation(
                            rt[:sz, f0 : f0 + fs], ps[:sz, :fs], ACTF.Relu,
                            scale=SCALE,
                        )

                    def softmax_ext(rin, sz=sz):
                        """in-place softmax over cols [0,N); col N rides along"""
                        mn = colp.tile([128, 1], F32, tag="mn")
                        nc.vector.tensor_reduce(
                            mn[:sz], rin[:sz, :N], axis=AX.X, op=OP.max,
                            negate=True,
                        )
                        acc = colp.tile([128, 1], F32, tag="acc")
                        nc.scalar.activation(
                            rin[:sz], rin[:sz], ACTF.Exp,
                            bias=mn[:sz], scale=1.0, accum_out=acc[:sz],
                        )
                        zf = colp.tile([128, 1], F32, tag="zf")
                        nc.vector.tensor_sub(zf[:sz], acc[:sz], rin[:sz, N : N + 1])
                        nc.gpsimd.normalize_recip(rin[:sz], rin[:sz], zf[:sz])

                    softmax_ext(r1)  # r1 -> a1
                    softmax_ext(r2)  # r2 -> a2
                    # l = (fcw0*a1 + fcw1*a2) + fcb  -- reference association
                    t2 = scrp.tile([128, N + 1], F32, tag="scr")
                    nc.scalar.activation(
                        t2[:sz], r2[:sz], ACTF.Copy, scale=fcw1[:sz]
                    )
                    nc.vector.scalar_tensor_tensor(
                        r1[:sz], r1[:sz], fcw0[:sz], t2[:sz],
                        op0=OP.mult, op1=OP.add,
                    )
                    nc.vector.tensor_scalar(
                        r1[:sz], r1[:sz], fcb[:sz], None, op0=OP.add
                    )
                    softmax_ext(r1)  # r1 -> adj
                    adj = r1
                    thr = adj[:sz, N : N + 1]
                    # ---- top-k mask, lowest-index tie breaking ----
                    gt = t2  # reuse
                    cnt = colp.tile([128, 1], F32, tag="cnt")
                    nc.vector.tensor_scalar(
                        gt[:sz, :N], adj[:sz, :N], thr, 0.0,
                        op0=OP.is_gt, op1=OP.add, accum_out=cnt[:sz],
                    )
                    eq = r2  # reuse
                    nc.vector.tensor_scalar(
                        eq[:sz, :N], adj[:sz, :N], thr, None, op0=OP.is_equal
                    )
                    # cum = cnt + prefix(eq); keep tie entries while cum <= K
                    cum = scrp.tile([128, N + 1], F32, tag="scr")
                    nc.vector.tensor_tensor_scan(
                        cum[:sz, :N], eq[:sz, :N], zeros[:sz, :N],
                        initial=cnt[:sz], op0=OP.add, op1=OP.add,
                    )
                    # eq <- (cum <= K)*eq ; then eq <- eq + gt ; adjB = adj*eq
                    nc.vector.scalar_tensor_tensor(
                        eq[:sz, :N], cum[:sz, :N], float(K), eq[:sz, :N],
                        op0=OP.is_le, op1=OP.mult,
                    )
                    nc.vector.tensor_add(eq[:sz, :N], eq[:sz, :N], gt[:sz, :N])
                    nc.vector.tensor_mul(
                        adjB[:sz, j, :N], adj[:sz, :N], eq[:sz, :N]
                    )

                # ========== stage C/D: diffusion (bf16) ====================
                for src, dst in ((hT, g1T), (g1T, g2T)):
                    for kk, (m0, msz) in enumerate(CH):
                        for f0, fs in _fch(CT):
                            ps = mmps.tile([128, 512], F32, tag="mm")
                            for j, (n0, sz) in enumerate(CH):
                                rhs = src[:sz, j].rearrange("p t c -> p (t c)")
                                nc.tensor.matmul(
                                    ps[:msz, :fs],
                                    adjB[:sz, j, m0 : m0 + msz],
                                    rhs[:, f0 : f0 + fs],
                                    start=(j == 0), stop=(j == NCH - 1),
                                )
                            dv = dst[:msz, kk].rearrange("p t c -> p (t c)")
                            if f0 == 512:
                                nc.vector.tensor_copy(dv[:, f0 : f0 + fs], ps[:msz, :fs])
                            else:
                                nc.scalar.activation(
                                    dv[:, f0 : f0 + fs], ps[:msz, :fs], ACTF.Copy
                                )

                # ========== stage E: projection + skip =====================
                for j, (n0, sz) in enumerate(CH):
                    gc1 = gcmp.tile([128, 128, T], BF16, tag="gc1")
                    gc2 = gcmp.tile([128, 128, T], BF16, tag="gc2")
                    for gsrc, gdst, eng in ((g1T, gc1, 0), (g2T, gc2, 1)):
                        for th in range(2):
                            tq = tpps.tile([128, T // 2, 128], BF16, tag="tp")
                            for tt in range(T // 2):
                                nc.tensor.transpose(
                                    tq[:, tt, :sz],
                                    gsrc[:sz, j, th * 6 + tt, :],
                                    identb[:sz, :sz],
                                )
                            src = tq[:, :, :sz].rearrange("p t m -> p m t")
                            dst = gdst[:, :sz, th * 6 : th * 6 + 6]
                            if (eng + th) % 2 == 0:
                                nc.scalar.activation(dst, src, ACTF.Copy)
                            else:
                                nc.vector.tensor_copy(dst, src)
                    g1v = gc1[:, :sz].rearrange("p n t -> p (n t)")
                    g2v = gc2[:, :sz].rearrange("p n t -> p (n t)")
                    ow = outwp.tile([128, CT], F32, tag="ow")
                    x2 = xinp.tile([128, CT], F32, tag="x2")
                    nc.sync.dma_start(x2[:, : sz * T], xf[:, n0 * T : (n0 + sz) * T])
                    for f0, fs in _fch(sz * T):
                        ps = mmps.tile([128, 512], F32, tag="mm")
                        nc.tensor.matmul(
                            ps[:, :fs], gcnbb[:], onesb[:, :fs],
                            start=True, stop=False,
                        )
                        nc.tensor.matmul(
                            ps[:, :fs], gw1T[:], g1v[:, f0 : f0 + fs],
                            start=False, stop=False,
                        )
                        nc.tensor.matmul(
                            ps[:, :fs], gw2T[:], g2v[:, f0 : f0 + fs],
                            start=False, stop=True,
                        )
                        # ow = (z + gcn_b)*emb + x  in one pass
                        nc.vector.scalar_tensor_tensor(
                            ow[:, f0 : f0 + fs], ps[:, :fs], embv[:],
                            x2[:, f0 : f0 + fs], op0=OP.mult, op1=OP.add,
                        )
                    nc.sync.dma_start(yf[:, n0 * T : (n0 + sz) * T], ow[:, : sz * T])
    nc.compile()
    return nc


_NC = None


def _get_nc():
    global _NC
    if _NC is None:
        _NC = build_nc()
    return _NC


def make_in_maps(inputs):
    x = np.ascontiguousarray(np.asarray(inputs["x"], dtype=np.float32))
    conv_w = np.asarray(inputs["conv_w"], np.float32)
    conv_b = np.asarray(inputs["conv_b"], np.float32)
    memory = np.ascontiguousarray(np.asarray(inputs["memory"], np.float32))
    fc_w = np.asarray(inputs["fc_w"], np.float32)
    fc_b = np.asarray(inputs["fc_b"], np.float32)
    gcn_w = np.asarray(inputs["gcn_w"], np.float32)
    gcn_b = np.asarray(inputs["gcn_b"], np.float32)
    emb = np.asarray(inputs["emb"], np.float32).reshape(C)

    shared = {
        "convwT": np.ascontiguousarray(conv_w.T),
        "convwTb": np.ascontiguousarray(conv_w.T).astype(ml_dtypes.bfloat16),
        "convbp": conv_b.reshape(C, 1).copy(),
        "convb12p": (T * conv_b).reshape(C, 1).copy(),
        "memory": memory,
        "fcw0": np.full((C, 1), fc_w[0, 0], np.float32),
        "fcw1": np.full((C, 1), fc_w[0, 1], np.float32),
        "fcb": np.full((C, 1), fc_b[0], np.float32),
        "gw1T": np.ascontiguousarray(gcn_w[:, :C].T).astype(ml_dtypes.bfloat16),
        "gw2T": np.ascontiguousarray(gcn_w[:, C:].T).astype(ml_dtypes.bfloat16),
        "gcnbb": gcn_b.reshape(1, C).astype(ml_dtypes.bfloat16),
        "emb": emb.reshape(C, 1).copy(),
    }
    return [
        {"x": np.ascontiguousarray(x[c * SPC : (c + 1) * SPC]), **shared}
        for c in range(NCORES)
    ]


def kernel(**inputs) -> np.ndarray:
    nc = _get_nc()
    in_maps = make_in_maps(inputs)
    res = run_bass_kernel_spmd(nc, in_maps, list(range(NCORES)))
    outs = [res.results[c]["y"] for c in range(NCORES)]
    return np.concatenate(outs, axis=0).astype(np.float32)
